# revision 1
# baseline (speedup 1.0000x reference)
"""Multi-head attention (RoPE, causal) Trainium2 kernel, SPMD over 8 NeuronCores.

Problem: x[2,2048,1024] @ {W_q,W_k,W_v}[1024,1024] -> 16-head causal attention
with RoPE -> @ W_o[1024,1024].

Sharding (batch x heads): core c handles batch b=c//4 and head group g=c%4
(4 heads = 256 of the 1024 qkv dims). Each core computes its heads' QKV
projections, RoPE, causal attention, and a partial out-projection
(ctx_g @ W_o[256g:256g+256, :]). The host sums the 4 partials per batch
(unshard of a partial-sum sharding) and transposes back.

On-device layout is fully transposed ([feature, seq]) so no transposes are
needed anywhere: scores are computed as scoresT[k,q] = K^T.T @ Q^T, the
softmax denominator falls out of the AV matmul via a ones-column appended to
V, and the out-projection consumes ctxT directly.

The whole kernel is one software pipeline over seq blocks sb:
  qk-proj(cc0) -> qk-proj(cc1) -> v-proj -> attention(cc0) -> attention(cc1)
  -> [next sb's qk-proj(cc0) covers the softmax-denominator reciprocal
     chain] -> normalize + partial out-projection + store for qb=sb.
Scores for the two heads of a chunk run concurrently in the two 64-row
groups of the PE array into one [128,1024] PSUM tile, so a single ACTIVATE
exponentiates both heads. Softmax normalization is deferred: unnormalized
ctxT and denominators are staged to SBUF; per (head-pair, query-block) one
DMA repartitions the denominators to [8,128] (reciprocal cost scales with
free size only), and stride-0 DMAs from a DRAM bounce broadcast the
reciprocals across partitions. Throwaway matmuls warm the PE's HAM clock
gate during the input load and through the final normalization chain.
"""

import numpy as np
import ml_dtypes

B = 2
S = 2048
D = 1024
H = 16
HD = 64
N_CORES = 8
H_PER_CORE = 4
DQ = H_PER_CORE * HD  # 256 qkv dims per core
N_DC = D // 128  # 8 contraction chunks
N_SB = S // 512  # 4 seq blocks of 512
N_KB = S // 128  # 16 key blocks of 128
THETA = 10000.0

_CACHED = None


def _build_kernel():
    import concourse.bass as bass
    import concourse.mybir as mybir
    import concourse.tile as tile
    from concourse import bacc

    f32 = mybir.dt.float32
    bf16 = mybir.dt.bfloat16

    nc = bacc.Bacc(None, target_bir_lowering=False, num_devices=N_CORES)

    xT = nc.dram_tensor("xT", [D, S], bf16, kind="ExternalInput")
    wq = nc.dram_tensor("wq", [D, DQ], bf16, kind="ExternalInput")
    wk = nc.dram_tensor("wk", [D, DQ], bf16, kind="ExternalInput")
    wv = nc.dram_tensor("wv", [D, DQ], bf16, kind="ExternalInput")
    wo = nc.dram_tensor("wo", [DQ, D], bf16, kind="ExternalInput")
    cosT = nc.dram_tensor("cosT", [128, S], f32, kind="ExternalInput")
    sinT = nc.dram_tensor("sinT", [128, S], f32, kind="ExternalInput")
    # masks[k, 1024*j + 512*h + q] = 1.0 if (128*j + k) <= q else 0 (h=0,1 same)
    masks = nc.dram_tensor("masks", [128, 4 * 1024], bf16, kind="ExternalInput")
    yT = nc.dram_tensor("yT", [D, S], bf16, kind="ExternalOutput")

    with tile.TileContext(nc) as tc:
        with (
            tc.tile_pool(name="persist", bufs=1) as persist,
            tc.tile_pool(name="attn", bufs=8) as attn_pool,
            tc.tile_pool(name="rope", bufs=4) as rope_pool,
            tc.tile_pool(name="small", bufs=4) as small_pool,
            tc.tile_pool(name="yout", bufs=3) as yout_pool,
            tc.tile_pool(name="dram", bufs=1, space="DRAM") as dram_pool,
            tc.tile_pool(name="psA", bufs=2, space="PSUM") as psA,  # scores 2-bank
            tc.tile_pool(name="psB", bufs=2, space="PSUM") as psB,  # ctx accum
            tc.tile_pool(name="psC", bufs=2, space="PSUM") as psC,  # proj/y
        ):
            # ---------------- input DMA ----------------
            # few, large DMAs: each dma_start costs ~600ns of queue issue
            wq_sb = persist.tile([128, N_DC, DQ], bf16, tag="wq")
            nc.sync.dma_start(
                out=wq_sb[:], in_=wq.rearrange("(c p) n -> p c n", p=128)
            )
            xt_sb = [
                persist.tile([128, S], bf16, tag=f"xt{dc}", name=f"xt{dc}")
                for dc in range(N_DC)
            ]
            for dc in range(N_DC):
                eng = nc.sync if dc % 2 == 0 else nc.gpsimd
                eng.dma_start(
                    out=xt_sb[dc][:], in_=xT[128 * dc : 128 * (dc + 1), :]
                )
            wk_sb = persist.tile([128, N_DC, DQ], bf16, tag="wk")
            nc.sync.dma_start(
                out=wk_sb[:], in_=wk.rearrange("(c p) n -> p c n", p=128)
            )
            cos_sb = persist.tile([128, S], f32, tag="cos")
            sin_sb = persist.tile([128, S], f32, tag="sin")
            nc.sync.dma_start(out=cos_sb[:, 0:512], in_=cosT[:, 0:512])
            nc.sync.dma_start(out=sin_sb[:, 0:512], in_=sinT[:, 0:512])
            wv_sb = persist.tile([128, N_DC, DQ], bf16, tag="wv")
            nc.sync.dma_start(
                out=wv_sb[:], in_=wv.rearrange("(c p) n -> p c n", p=128)
            )
            nc.sync.dma_start(out=cos_sb[:, 512:S], in_=cosT[:, 512:S])
            nc.sync.dma_start(out=sin_sb[:, 512:S], in_=sinT[:, 512:S])
            mask_sb = persist.tile([128, 4 * 1024], bf16, tag="mask")
            nc.sync.dma_start(out=mask_sb[:], in_=masks[:])
            wo_sb = persist.tile([128, 2, D], bf16, tag="wo")
            nc.sync.dma_start(
                out=wo_sb[:], in_=wo.rearrange("(c p) n -> p c n", p=128)
            )

            # PE warm-up: the HAM clock gate needs ~3.4us of sustained
            # activity to lift the PE to 2.4GHz; run throwaway matmuls on the
            # first-arrived weight tile while x is still streaming in
            warm0 = psA.tile([128, DQ], f32, tag="score", name="warm0")
            for wi in range(24):
                nc.tensor.matmul(
                    warm0[:],
                    wq_sb[:, 0, 0:128],
                    wq_sb[:, wi % 4, :],
                    start=True,
                    stop=True,
                )

            # persistent intermediates
            qT_sb = persist.tile([128, 2, S], bf16, tag="qT")  # [64h..., cc, s]
            kT_sb = persist.tile([128, 2, S], bf16, tag="kT")
            v_sb = persist.tile([128, N_KB, H_PER_CORE, HD + 1], bf16, tag="v")
            nc.vector.memset(v_sb[:, :, :, HD : HD + 1], 1.0)
            ctxT_sb = persist.tile([128, 2, S], bf16, tag="ctxT")  # unnormalized
            # denominators staged on one partition (engine writes must start at
            # partition 0/32/64/96); chunk qb*4+hh holds head hh, block qb
            stage_sb = persist.tile([1, H_PER_CORE * S], f32, tag="stage")
            recip_dram = dram_pool.tile([N_SB, H_PER_CORE, 512], bf16, tag="rdram")

            # ---------------- helpers ----------------
            def rope(src_ps, dst_sb, cc, sb):
                """dst = src*cos + rotate_half(src)*sin, fp32 in, bf16 out.

                The rotate-half partition shift is done by small SBUF->SBUF
                DMAs (a [32,512] DVE op costs as much as a [128,512] one, so
                quarter-sized DVE ops waste 3/4 of the lanes; DMA engines are
                otherwise idle).
                """
                ss = slice(512 * sb, 512 * (sb + 1))
                t1 = rope_pool.tile([128, 512], bf16, tag="ropeA", name="t1")
                nc.vector.tensor_mul(t1[:], src_ps[:], cos_sb[:, ss])
                # sin table is pre-shifted on the host (sinx[p] =
                # sin_signed[partner(p)]) so this product is computed at the
                # SOURCE rows and only then moved to the partner rows by DMA
                t2p = rope_pool.tile([128, 512], bf16, tag="ropeQ", name="t2p")
                nc.vector.tensor_mul(t2p[:], src_ps[:], sin_sb[:, ss])
                rot = rope_pool.tile([128, 512], bf16, tag="ropeB", name="rot")
                for quarter in range(4):
                    o = 32 * quarter
                    src_o = o + 32 if quarter % 2 == 0 else o - 32
                    nc.gpsimd.dma_start(
                        out=rot[o : o + 32, :], in_=t2p[src_o : src_o + 32, :]
                    )
                nc.vector.tensor_add(dst_sb[:, cc, ss], t1[:], rot[:])

            def proj_qk(cc, sb):
                ss = slice(512 * sb, 512 * (sb + 1))
                q_ps = psC.tile([128, 512], f32, tag="proj", name="q_ps")
                for dc in range(N_DC):
                    nc.tensor.matmul(
                        q_ps[:],
                        wq_sb[:, dc, 128 * cc : 128 * (cc + 1)],
                        xt_sb[dc][:, ss],
                        start=(dc == 0),
                        stop=(dc == N_DC - 1),
                    )
                rope(q_ps, qT_sb, cc, sb)
                k_ps = psC.tile([128, 512], f32, tag="proj", name="k_ps")
                for dc in range(N_DC):
                    nc.tensor.matmul(
                        k_ps[:],
                        wk_sb[:, dc, 128 * cc : 128 * (cc + 1)],
                        xt_sb[dc][:, ss],
                        start=(dc == 0),
                        stop=(dc == N_DC - 1),
                    )
                rope(k_ps, kT_sb, cc, sb)

            def proj_v(sc):
                v_ps = psC.tile([128, DQ], f32, tag="proj", name="v_ps")
                for dc in range(N_DC):
                    nc.tensor.matmul(
                        v_ps[:],
                        xt_sb[dc][:, 128 * sc : 128 * (sc + 1)],
                        wv_sb[:, dc, :],
                        start=(dc == 0),
                        stop=(dc == N_DC - 1),
                    )
                nc.vector.tensor_copy(
                    v_sb[:, sc, :, 0:HD],
                    v_ps[:].rearrange("p (h d) -> p h d", h=H_PER_CORE),
                )

            def attention(cc, qb, filler=None):
                """Causal attention for head pair cc, query block qb.

                Per k-block: two score matmuls (head h in PE row-group h) into
                one [128,1024] PSUM tile, one exp over both heads, mask on
                diagonal blocks, then (one k-block delayed) the two AV
                matmuls accumulating ctx+denominator via the ones column.

                `filler` is a list of callables emitting independent PE work,
                interleaved between k-blocks to cover pipeline bubbles.
                """
                qs = slice(512 * qb, 512 * (qb + 1))
                nkb = 4 * qb + 4
                filler = list(filler or [])
                ctx_ps = [
                    psB.tile([HD + 1, 512], f32, tag="ctx", name=f"ctx{h}")
                    for h in range(2)
                ]
                pending = None  # attnT tile whose AV matmuls haven't run
                for kb in range(nkb):
                    s_ps = psA.tile([128, 1024], f32, tag="score", name="s_ps")
                    for h in range(2):
                        hp = slice(64 * h, 64 * (h + 1))
                        nc.tensor.matmul(
                            s_ps[:, 512 * h : 512 * (h + 1)],
                            kT_sb[hp, cc, 128 * kb : 128 * (kb + 1)],
                            qT_sb[hp, cc, qs],
                            start=True,
                            stop=True,
                        )
                    a_t = attn_pool.tile([128, 1024], bf16, tag="attnT", name="a_t")
                    nc.scalar.activation(
                        a_t[:],
                        s_ps[:],
                        mybir.ActivationFunctionType.Exp,
                        scale=float(1.0 / np.sqrt(HD)),
                    )
                    if kb >= 4 * qb:
                        j = kb - 4 * qb
                        nc.vector.tensor_mul(
                            a_t[:], a_t[:], mask_sb[:, 1024 * j : 1024 * (j + 1)]
                        )
                    if pending is not None:
                        pkb, p_t = pending
                        for h in range(2):
                            nc.tensor.matmul(
                                ctx_ps[h][:],
                                v_sb[:, pkb, 2 * cc + h, :],
                                p_t[:, 512 * h : 512 * (h + 1)],
                                start=(pkb == 0),
                                stop=False,
                            )
                    pending = (kb, a_t)
                pkb, p_t = pending
                for h in range(2):
                    nc.tensor.matmul(
                        ctx_ps[h][:],
                        v_sb[:, pkb, 2 * cc + h, :],
                        p_t[:, 512 * h : 512 * (h + 1)],
                        start=(pkb == 0),
                        stop=True,
                    )
                # stage denominators first (the normalization chain hangs
                # off them), then independent PE filler work to cover the
                # chain, then the bulk ctx copies
                r0 = qb * H_PER_CORE + 2 * cc
                nc.vector.tensor_copy(
                    stage_sb[0:1, 512 * r0 : 512 * (r0 + 1)],
                    ctx_ps[0][HD : HD + 1, :],
                )
                nc.scalar.copy(
                    stage_sb[0:1, 512 * (r0 + 1) : 512 * (r0 + 2)],
                    ctx_ps[1][HD : HD + 1, :],
                )
                for f in filler:
                    f()
                for h in range(2):
                    nc.vector.tensor_copy(
                        ctxT_sb[64 * h : 64 * (h + 1), cc, qs], ctx_ps[h][0:HD, :]
                    )

            def normalize(cc, qb):
                """Reciprocal + broadcast + scale for head pair cc, block qb."""
                # repartition [1, 1024] -> [8, 128] so reciprocal is cheap
                # (reciprocal cost scales with free size only)
                base = (qb * H_PER_CORE + 2 * cc) * 512
                den_q = small_pool.tile([8, 128], f32, tag="den_q", name="den_q")
                nc.sync.dma_start(
                    out=den_q[:], in_=stage_sb[0:1, base : base + 1024]
                )
                rec_q = small_pool.tile([8, 128], bf16, tag="rec_q", name="rec_q")
                with nc.allow_low_precision(
                    reason="bf16 softmax denom matches bf16 attn weights"
                ):
                    nc.vector.reciprocal(rec_q[:], den_q[:])
                if cc == 1 and qb == N_SB - 1:
                    # keep the PE's HAM clock warm through the tail
                    # normalization chain: a few scratch matmuls gated on the
                    # chain's own data so the scheduler cannot hoist them
                    warm = psA.tile([128, 512], f32, tag="score", name="warm")
                    for wi in range(8):
                        nc.tensor.matmul(
                            warm[:],
                            rec_q[:],
                            xt_sb[wi][0:8, 0:512],
                            start=True,
                            stop=True,
                        )
                nc.sync.dma_start(
                    out=recip_dram[qb, 2 * cc : 2 * cc + 2, :], in_=rec_q[:]
                )
                qs = slice(512 * qb, 512 * (qb + 1))
                bc_sb = small_pool.tile([128, 512], bf16, tag="bcast", name="bc_sb")
                for h in range(2):
                    row = recip_dram[qb, 2 * cc + h, :]
                    bcast = bass.AP(
                        tensor=row.tensor,
                        offset=row.offset,
                        ap=[[0, 64]] + list(row.ap)[-1:],
                    )
                    nc.sync.dma_start(
                        out=bc_sb[64 * h : 64 * (h + 1), :], in_=bcast
                    )
                nc.vector.tensor_mul(
                    ctxT_sb[:, cc, qs], ctxT_sb[:, cc, qs], bc_sb[:]
                )

            def out_proj(qb, ocs):
                qs = slice(512 * qb, 512 * (qb + 1))
                for oc in ocs:
                    y_ps = psC.tile([128, 512], f32, tag="proj", name="y_ps")
                    for cc in range(2):
                        nc.tensor.matmul(
                            y_ps[:],
                            wo_sb[:, cc, 128 * oc : 128 * (oc + 1)],
                            ctxT_sb[:, cc, qs],
                            start=(cc == 0),
                            stop=(cc == 1),
                        )
                    y_sb = yout_pool.tile([128, 512], bf16, tag="y", name="y_sb")
                    nc.vector.tensor_copy(y_sb[:], y_ps[:])
                    nc.sync.dma_start(
                        out=yT[128 * oc : 128 * (oc + 1), qs], in_=y_sb[:]
                    )

            # ---------------- main pipeline ----------------
            proj_qk(0, 0)
            for sb in range(N_SB):
                proj_qk(1, sb)
                for sc in range(4 * sb, 4 * sb + 4):
                    proj_v(sc)
                attention(0, sb)
                normalize(0, sb)  # chain covered by attention(1, sb) PE work
                if sb == N_SB - 1:
                    attention(
                        1,
                        sb,
                        filler=[
                            (lambda oc=oc: out_proj(2, [oc]))
                            for oc in range(4, N_DC)
                        ],
                    )
                else:
                    attention(1, sb)
                normalize(1, sb)
                if sb < N_SB - 1:
                    # emit the next block's first projection before this
                    # block's out-projection so the reciprocal chain is
                    # covered by PE work and the PE never idles
                    proj_qk(0, sb + 1)
                if sb < 2:
                    out_proj(sb, range(N_DC))
                elif sb == 2:
                    # hold back half of qb=2's out-projection; it is emitted
                    # as filler inside attention(1, 3) to cover the final
                    # normalization chain
                    out_proj(2, range(0, 4))
                else:
                    out_proj(3, range(N_DC))

    nc.compile()
    return nc


def _rope_tables():
    inv_freq = (
        1.0 / (THETA ** (np.arange(0, HD, 2, dtype=np.float32) / HD))
    ).astype(np.float32)
    pos = np.arange(S, dtype=np.float32)
    ang = pos[:, None] * inv_freq[None, :]  # [S, 32]
    cos_half = np.cos(ang).astype(np.float32).T  # [32, S]
    sin_half = np.sin(ang).astype(np.float32).T
    # per-head 64 rows: cos rows duplicated. The sin table is PRE-SHIFTED:
    # row p holds sin_signed[partner(p)] (partner = rotate-half swap), so the
    # kernel multiplies at the source rows and a plain partition-shift DMA
    # finishes rotate-half: sinx per head = (+sin | -sin).
    cos64 = np.concatenate([cos_half, cos_half], axis=0)
    sinx64 = np.concatenate([sin_half, -sin_half], axis=0)
    cosT = np.concatenate([cos64, cos64], axis=0)  # [128, S] two heads
    sinT = np.concatenate([sinx64, sinx64], axis=0)
    return np.ascontiguousarray(cosT), np.ascontiguousarray(sinT)


def _masks():
    k = np.arange(128)[:, None]
    q = np.arange(512)[None, :]
    m = np.empty((128, 4 * 1024), dtype=ml_dtypes.bfloat16)
    for j in range(4):
        blk = (128 * j + k <= q).astype(ml_dtypes.bfloat16)
        m[:, 1024 * j : 1024 * j + 512] = blk
        m[:, 1024 * j + 512 : 1024 * (j + 1)] = blk
    return m


def kernel(x, W_q, W_k, W_v, W_o):
    global _CACHED
    from concourse.bass_utils import run_bass_kernel_spmd

    if _CACHED is None:
        _CACHED = _build_kernel()
    nc = _CACHED

    bf = ml_dtypes.bfloat16
    cosT, sinT = _rope_tables()
    masks = _masks()
    x = np.asarray(x)
    W_q, W_k, W_v, W_o = (np.asarray(w) for w in (W_q, W_k, W_v, W_o))
    xT = [np.ascontiguousarray(x[b].T).astype(bf) for b in range(B)]

    in_maps = []
    for c in range(N_CORES):
        b, g = divmod(c, 4)
        cols = slice(DQ * g, DQ * (g + 1))
        in_maps.append(
            {
                "xT": xT[b],
                "wq": np.ascontiguousarray(W_q[:, cols]).astype(bf),
                "wk": np.ascontiguousarray(W_k[:, cols]).astype(bf),
                "wv": np.ascontiguousarray(W_v[:, cols]).astype(bf),
                "wo": np.ascontiguousarray(W_o[cols, :]).astype(bf),
                "cosT": cosT,
                "sinT": sinT,
                "masks": masks,
            }
        )

    res = run_bass_kernel_spmd(nc, in_maps, core_ids=list(range(N_CORES)))
    kernel.last_results = res

    y = np.empty((B, S, D), dtype=np.float32)
    for b in range(B):
        acc = res.results[4 * b]["yT"].astype(np.float32)
        for g in range(1, 4):
            acc += res.results[4 * b + g]["yT"].astype(np.float32)
        y[b] = acc.T
    return y

